# revision 1
# baseline (speedup 1.0000x reference)
"""Trainium2 Bass kernel for nn_Attention pooling module.

Math (per batch b):
    seq_h  = seq @ w1.T + b1                      [S, 96]
    z      = seq_h + (tgt @ w2.T + b2)            [S, 96]  (tgt term broadcast over S)
    scores = sigmoid(z).sum(-1)                   [S]
    scores = where(mask, -1e9, scores)
    attn   = softmax(scores)                      [S]
    out    = attn @ seq_h = (attn @ seq) @ w1.T + b1      (since sum(attn) == 1)

Distribution: pure data-parallel over the batch dim (2048 -> 8 x 256).

Device-side layout (per core, bf16 data, PANEL = 128 batches):
  - seq uploaded twice in host-packed layouts:
      * "seqt": d-major [group, 98, 8*512]  (rows 96/97 = even/odd-batch indicator
        rows so the per-batch bias folds into the projection as K=98 contraction)
      * "natg": s-major [group, 128, 8*4*96] for pooling
  - proj per batch-pair: z2 = lhsT.T @ seqt_aug -> PSUM [96, 1024]
    (lhsT = [w1.T ; bias_b ; bias_b+1], persistent tiles, bias rows DMA'd per pair)
  - sigmoid on ScalarE per pair [96, 1024] -> bf16 SBUF
  - scores: per batch, one-hot-window matmul accumulating PSUM [128, 512]
  - masked softmax over free dim, ACT exp with accum_out denominator
  - attn transposed via PE -> attnT [128s, 128b]
  - pooling: per (4-batch group, chunk) one matmul lhsT=attnT_chunk [128,128],
    rhs = 4 nat chunks [128, 384] -> PSUM [128, 384]; the 4 valid diagonal
    [1, 96] blocks extracted by DVE into pooled [128b, 96d]
  - final projection: PE transpose + augmented matmul (bias row folded), fp32
"""

from contextlib import ExitStack

import numpy as np
import ml_dtypes

import concourse.bass as bass
import concourse.bacc as bacc
import concourse.tile as tile
from concourse import mybir
from concourse.bass_utils import run_bass_kernel_spmd

BF16 = mybir.dt.bfloat16
F32 = mybir.dt.float32
NP_BF16 = ml_dtypes.bfloat16

N_CORES = 8
B = 2048
S = 512
D = 96
BC = B // N_CORES      # 256 batches per core
PANEL = 128            # batches per softmax panel
GROUP = 8              # batches per DMA group
NPANEL = BC // PANEL   # 2
NGROUP = BC // GROUP   # 32
GPP = PANEL // GROUP   # groups per panel = 16
NCHUNK = S // 128      # 4 token chunks of 128
PG = 4                 # batches per pooling cross-product matmul

Sigmoid = mybir.ActivationFunctionType.Sigmoid
Exp = mybir.ActivationFunctionType.Exp


def build_program() -> bass.Bass:
    nc = bacc.Bacc(
        "TRN2", target_bir_lowering=False, debug=False, num_devices=N_CORES
    )

    seqt_d = nc.dram_tensor(
        "seqt", [NGROUP, D + GROUP, GROUP * S], BF16, kind="ExternalInput"
    )
    natg_d = nc.dram_tensor(
        "natg", [NGROUP, 128, GROUP * NCHUNK * D], BF16, kind="ExternalInput"
    )
    brow_d = nc.dram_tensor("brow", [BC, D], BF16, kind="ExternalInput")
    maskneg_d = nc.dram_tensor("maskneg", [BC, S], F32, kind="ExternalInput")
    w1t_d = nc.dram_tensor("w1t", [D, D], BF16, kind="ExternalInput")
    w1aug_d = nc.dram_tensor("w1aug", [D + 1, D], F32, kind="ExternalInput")
    zbuf_d = nc.dram_tensor("zbuf", [D, 2 * PANEL - 1], BF16, kind="ExternalInput")
    ident_d = nc.dram_tensor("ident", [128, 128], BF16, kind="ExternalInput")
    identf_d = nc.dram_tensor("identf", [128, 128], F32, kind="ExternalInput")
    out_d = nc.dram_tensor("out", [BC, D], F32, kind="ExternalOutput")
    # per-panel scratch for the pooling diagonal bounce: [group, 4, 4*96]
    poolscr_d = nc.dram_tensor(
        "poolscr", [NPANEL, PANEL // PG, PG, PG * D], F32
    )

    with tile.TileContext(nc) as tc, ExitStack() as ctx:
        const_pool = ctx.enter_context(tc.tile_pool(name="const", bufs=1))
        natp = ctx.enter_context(tc.tile_pool(name="natp", bufs=18))
        seqp = ctx.enter_context(tc.tile_pool(name="seqp", bufs=3))
        sigp = ctx.enter_context(tc.tile_pool(name="sigp", bufs=3))
        smp = ctx.enter_context(tc.tile_pool(name="smp", bufs=2))
        zpsum = ctx.enter_context(
            tc.tile_pool(name="zpsum", bufs=2, space=bass.MemorySpace.PSUM)
        )
        spsum = ctx.enter_context(
            tc.tile_pool(name="spsum", bufs=1, space=bass.MemorySpace.PSUM)
        )
        ppsum = ctx.enter_context(
            tc.tile_pool(name="ppsum", bufs=2, space=bass.MemorySpace.PSUM)
        )
        tpsum = ctx.enter_context(
            tc.tile_pool(name="tpsum", bufs=1, space=bass.MemorySpace.PSUM)
        )

        w1aug_sb = const_pool.tile([D + 1, D], F32)
        nc.sync.dma_start(w1aug_sb[:], w1aug_d[:])
        zbuf_sb = const_pool.tile([D, 2 * PANEL - 1], BF16)
        nc.sync.dma_start(zbuf_sb[:], zbuf_d[:])
        ident_sb = const_pool.tile([128, 128], BF16)
        nc.sync.dma_start(ident_sb[:], ident_d[:])
        identf_sb = const_pool.tile([128, 128], F32)
        nc.sync.dma_start(identf_sb[:], identf_d[:])
        # Two persistent projection lhsT tiles [w1.T ; bias rows for the
        # current 8-batch group], alternated across groups; w1.T rows written
        # once, the 8 bias rows DMA'd per group on the scalar HWDGE ring.
        lhsT_tiles = [
            const_pool.tile([D + GROUP, D], BF16, name=f"lhsT{k}", tag=f"lhsT{k}")
            for k in (0, 1)
        ]
        for t in lhsT_tiles:
            nc.scalar.dma_start(t[0:D, :], w1t_d[:])

        for panel in range(NPANEL):
            # ---------------- Phase A: scores for PANEL batches ----------------
            scores_ps = spsum.tile([PANEL, S], F32)
            nat_tiles = []
            for gi in range(GPP):
                g = panel * GPP + gi
                natt = natp.tile([128, GROUP * NCHUNK * D], BF16)
                nc.sync.dma_start(natt[:], natg_d[g])
                nat_tiles.append(natt)
                seqt_sb = seqp.tile([D + GROUP, GROUP * S], BF16)
                nc.sync.dma_start(seqt_sb[:], seqt_d[g])
                lt = lhsT_tiles[g % 2]
                nc.scalar.dma_start(
                    lt[D : D + GROUP, :], brow_d[g * GROUP : (g + 1) * GROUP, :]
                )
                for i in range(0, GROUP, 2):
                    bl = gi * GROUP + i           # panel-local batch index
                    z_ps = zpsum.tile([D, 2 * S], F32)
                    for k in (0, 1):
                        nc.tensor.matmul(
                            z_ps[:, k * S : (k + 1) * S],
                            lt[:],
                            seqt_sb[:, (i + k) * S : (i + k + 1) * S],
                            start=True,
                            stop=True,
                        )
                    sig_sb = sigp.tile([D, 2 * S], BF16)
                    nc.scalar.activation(sig_sb[:], z_ps[:], Sigmoid)
                    for k in (0, 1):
                        nc.tensor.matmul(
                            scores_ps[:],
                            zbuf_sb[:, PANEL - 1 - (bl + k) : 2 * PANEL - 1 - (bl + k)],
                            sig_sb[:, k * S : (k + 1) * S],
                            start=(bl + k == 0),
                            stop=(bl + k == PANEL - 1),
                        )

            # ---------------- Phase B: masked softmax over s ----------------
            mneg = smp.tile([PANEL, S], F32, tag="mneg")
            nc.scalar.dma_start(
                mneg[:], maskneg_d[panel * PANEL : (panel + 1) * PANEL, :]
            )
            sc_sb = smp.tile([PANEL, S], F32, tag="scsb")
            nc.vector.tensor_add(sc_sb[:], scores_ps[:], mneg[:])
            nmx = smp.tile([PANEL, 1], F32, tag="nmx")
            nc.vector.reduce_max(
                nmx[:], sc_sb[:], axis=mybir.AxisListType.X, negate=True
            )
            expv = smp.tile([PANEL, S], BF16, tag="expv")
            ssum = smp.tile([PANEL, 1], F32, tag="ssum")
            nc.scalar.activation(
                expv[:], sc_sb[:], Exp, bias=nmx[:, 0:1], accum_out=ssum[:]
            )
            rsum = smp.tile([PANEL, 1], F32, tag="rsum")
            nc.vector.reciprocal(rsum[:], ssum[:])
            attn = smp.tile([PANEL, S], BF16, tag="attn")
            nc.vector.tensor_scalar_mul(attn[:], expv[:], rsum[:, 0:1])
            # attnT chunks padded to 256 cols (zeros) so the pooling lhsT can
            # slice [lo : lo+128] with the valid batch landing on row q=0..3
            attnT_sb = smp.tile([128, NCHUNK * (PANEL + 128)], BF16, tag="attnT")
            for j in range(NCHUNK):
                att_ps = tpsum.tile([128, PANEL], BF16, tag="tp")
                nc.tensor.transpose(
                    att_ps[:], attn[:, j * 128 : (j + 1) * 128], ident_sb[:]
                )
                nc.vector.tensor_copy(
                    attnT_sb[:, j * 256 : j * 256 + PANEL], att_ps[:]
                )
                nc.vector.memset(attnT_sb[:, j * 256 + PANEL : (j + 1) * 256], 0.0)

            # ---------------- Phase C: attn-weighted pooling ----------------
            # One cross-product matmul per (4-batch group, chunk): all 128
            # attn rows against 4 batches' nat chunks; rows != the matching
            # batch are garbage, the 4 diagonal [1, 96] blocks are extracted.
            for t in range(PANEL // PG):
                lo = t * PG                       # panel-local first batch
                gi, i0 = divmod(lo, GROUP)
                natt = nat_tiles[gi]
                pool_ps = ppsum.tile([PANEL, PG * D], F32)
                nat3 = natt[:].rearrange("p (i j d) -> p i j d", j=NCHUNK, d=D)
                for j in range(NCHUNK):
                    # shifted lhsT: row m <- attn batch lo+m, so the valid
                    # diagonal blocks land on rows 0..PG-1
                    nc.tensor.matmul(
                        pool_ps[:],
                        attnT_sb[:, j * 256 + lo : j * 256 + lo + 128],
                        nat3[:, i0 : i0 + PG, j, :],
                        start=(j == 0),
                        stop=(j == NCHUNK - 1),
                    )
                pout = smp.tile([PG, PG * D], F32, tag="pout")
                nc.vector.tensor_copy(pout[:], pool_ps[0:PG, :])
                nc.scalar.dma_start(poolscr_d[panel, t], pout[:])
            # gather the diagonal [1, 96] blocks back: batch 4t+q reads
            # scratch offset t*1536 + q*(384+96) + d
            pooled_sb = smp.tile([PANEL, D], F32, tag="pooled")
            base = poolscr_d[panel]
            diag_ap = bass.AP(
                tensor=base.tensor,
                offset=base.offset,
                ap=[[PG * PG * D, PANEL // PG], [(PG + 1) * D, PG], [1, D]],
            )
            nc.scalar.dma_start(pooled_sb[:], diag_ap)

            # ---------------- Phase D: final projection + bias ----------------
            pT_ps = tpsum.tile([D, PANEL], F32, tag="tp")
            nc.tensor.transpose(pT_ps[:], pooled_sb[:], identf_sb[:])
            paug = smp.tile([D + 1, PANEL], F32, tag="paug")
            nc.vector.tensor_copy(paug[0:D, :], pT_ps[:])
            nc.vector.memset(paug[D : D + 1, :], 1.0)
            outp_ps = tpsum.tile([PANEL, D], F32, tag="tp")
            nc.tensor.matmul(outp_ps[:], paug[:], w1aug_sb[:], start=True, stop=True)
            out_sb = smp.tile([PANEL, D], F32, tag="outsb")
            nc.scalar.copy(out_sb[:], outp_ps[:])
            nc.scalar.dma_start(
                out_d[panel * PANEL : (panel + 1) * PANEL, :], out_sb[:]
            )

    nc.compile()
    return nc


def prepare_in_maps(inputs: dict) -> list[dict]:
    seq = np.asarray(inputs["seq_item_embedding"], dtype=np.float32)
    tgt = np.asarray(inputs["target_item_embedding"], dtype=np.float32)
    mask = np.asarray(inputs["mask"])
    w1w = np.asarray(inputs["w1_weight"], dtype=np.float32)
    w1b = np.asarray(inputs["w1_bias"], dtype=np.float32)
    w2w = np.asarray(inputs["w2_weight"], dtype=np.float32)
    w2b = np.asarray(inputs["w2_bias"], dtype=np.float32)

    seq_bf = seq.astype(NP_BF16)
    bias_all = (tgt[:, 0, :] @ w2w.T + w2b + w1b).astype(np.float32)  # [B, D]
    maskneg = np.where(
        mask[:, :S, 0], np.float32(-1e9), np.float32(0.0)
    ).astype(np.float32)  # [B, S]

    w1t_bf = np.ascontiguousarray(w1w.T).astype(NP_BF16)
    w1aug_f = np.ascontiguousarray(
        np.concatenate([w1w.T, w1b[None, :]], axis=0).astype(np.float32)
    )
    zbuf_bf = np.zeros((D, 2 * PANEL - 1), dtype=NP_BF16)
    zbuf_bf[:, PANEL - 1] = 1.0
    ident_bf = np.eye(128, dtype=NP_BF16)
    ident_f = np.eye(128, dtype=np.float32)

    # indicator rows folded into the seqt upload: row 96+i is 1 exactly on
    # batch i's columns, selecting that batch's bias row of the lhsT
    ind = np.zeros((GROUP, GROUP * S), dtype=NP_BF16)
    for i in range(GROUP):
        ind[i, i * S : (i + 1) * S] = 1.0

    in_maps = []
    for c in range(N_CORES):
        sl = slice(c * BC, (c + 1) * BC)
        sc = seq_bf[sl]  # [BC, S, D]
        seqt_core = np.ascontiguousarray(
            sc.reshape(NGROUP, GROUP, S, D).transpose(0, 3, 1, 2)
        ).reshape(NGROUP, D, GROUP * S)
        seqt = np.concatenate(
            [seqt_core, np.broadcast_to(ind, (NGROUP, GROUP, GROUP * S))], axis=1
        )
        natg = np.ascontiguousarray(
            sc.reshape(NGROUP, GROUP, NCHUNK, 128, D).transpose(0, 3, 1, 2, 4)
        ).reshape(NGROUP, 128, GROUP * NCHUNK * D)
        in_maps.append(
            {
                "seqt": np.ascontiguousarray(seqt),
                "natg": natg,
                "brow": np.ascontiguousarray(bias_all[sl]).astype(NP_BF16),
                "maskneg": np.ascontiguousarray(maskneg[sl]),
                "w1t": w1t_bf,
                "w1aug": w1aug_f,
                "zbuf": zbuf_bf,
                "ident": ident_bf,
                "identf": ident_f,
            }
        )
    return in_maps


_CACHED_NC = None


def run(inputs: dict, trace: bool = False, tmpdir: str | None = None):
    global _CACHED_NC
    in_maps = prepare_in_maps(inputs)
    if _CACHED_NC is None:
        _CACHED_NC = build_program()
    res = run_bass_kernel_spmd(
        _CACHED_NC, in_maps, list(range(N_CORES)), trace=trace, tmpdir=tmpdir
    )
    out = np.concatenate([r["out"] for r in res.results], axis=0)
    return out, res


def kernel(**inputs) -> np.ndarray:
    out, _ = run(inputs, trace=False)
    return out

